# revision 22
# baseline (speedup 1.0000x reference)
"""Single-head attention (B=8, S=2048, D=384) on 8 NeuronCores.

Sharding: data-parallel over batch — core b computes batch element b
entirely (QKV projections + softmax(Q K^T) V), weights replicated.

Host-side marshalling (part of kernel()'s sharding step): x is fed
pre-transposed per core as xT [D, S], and the weights pre-transposed as
WqT/WkT/WvT [D, D] (zero host FLOPs — layout only). This removes every
PE-transpose from the device kernel.

Per-core dataflow (all on one NeuronCore, f32 in/out):
  - QT = (WqT)^T-blocks @ xT and KT likewise (both [D, S], feature-major),
    V = x @ Wv^T in natural [S, D] layout — all from xT via the PE, with a
    ones column pair appended to V -> vA [S, D+2].
  - scores^T tile alphaT[k, q] = KT-block^T @ QT-block accumulated over the
    3 e-tiles; exp() on ScalarE (no max subtraction needed: logits are
    ~N(0, 42), |logit| < ~45 << 88, so fp32 exp cannot overflow; softmax is
    shift-invariant so the result matches the reference's max-subtracted
    computation up to rounding).
  - out_raw[q, :D] and the softmax denominator accumulate TOGETHER via
    out_acc[q, 0:D+2] += expT[k, q-block]^T @ vA[k-block, :]  (the ones
    columns of vA make column D equal sum_k exp) — no cross-partition
    reduction ever needed.
  - out[q, e] = out_raw[q, e] * (1 / out_acc[q, D]).

Matmuls run as float32r (full PE rate at N>=256); fp32 PSUM accumulation.
"""

import os
import numpy as np

import concourse.bass as bass
import concourse.bacc as bacc
import concourse.tile as tile
from concourse import mybir
from concourse import bass_utils

P = 128          # partitions / PE tile edge
S = 2048         # sequence length per core
D = 384          # model dim
NB = 8           # batch == number of cores
DT = D // P      # 3 feature tiles
ST = S // P      # 16 sequence tiles
QC = 512         # q-column chunk (PSUM bank of f32)
NQ = S // QC     # 4 q chunks
F32 = mybir.dt.float32
F32R = mybir.dt.float32r
BF16 = mybir.dt.bfloat16

# "f32r" (default), "f32", or "bf16" — matmul operand precision.
MM_MODE = os.environ.get("ATT_MM_MODE", "f32r")
# 1: DMA straight into f32r operand tiles; 0: DMA to f32 staging + DVE cast
DIRECT = os.environ.get("ATT_DIRECT", "1") == "1"


def _build():
    sb_dt = {"f32r": F32R, "bf16": BF16, "f32": F32}[MM_MODE]

    nc = bacc.Bacc(
        "TRN2", target_bir_lowering=False, debug=False, enable_asserts=False
    )
    # DRAM inputs carry the matmul dtype so the direct DMA is cast-free
    # (float32r has identical 4-byte layout; numpy side stays float32)
    in_dt = {"f32r": F32R, "bf16": F32, "f32": F32}[MM_MODE] if DIRECT else F32
    xt = nc.dram_tensor("xt", [D, S], in_dt, kind="ExternalInput").ap()
    wqt = nc.dram_tensor("wqt", [D, D], in_dt, kind="ExternalInput").ap()
    wkt = nc.dram_tensor("wkt", [D, D], in_dt, kind="ExternalInput").ap()
    wvt = nc.dram_tensor("wvt", [D, D], in_dt, kind="ExternalInput").ap()
    out = nc.dram_tensor("out", [S, D], F32, kind="ExternalOutput").ap()

    with tile.TileContext(nc) as tc:
        with (
            tc.tile_pool(name="const", bufs=1) as const_pool,
            tc.tile_pool(name="big", bufs=1) as big,
            tc.tile_pool(name="stage", bufs=4) as stage_pool,
            tc.tile_pool(name="expool", bufs=3) as ex_pool,
            tc.tile_pool(name="obpool", bufs=3) as ob_pool,
            tc.tile_pool(name="smalls", bufs=4) as small_pool,
            tc.tile_pool(name="ps_stage", bufs=4, space="PSUM") as ps_stage,
            tc.tile_pool(name="ps_acc", bufs=4, space="PSUM") as ps_acc,
        ):
            ones_c = const_pool.tile([P, 2], F32, tag="ones", name="ones_c")
            nc.vector.memset(ones_c, 1.0)

            # HAM warmup: keep the PE busy during the DMA head so the clock
            # gate is at 8/8 when the first real matmul issues (~3.4us of
            # sustained activity flips it; idle PE starts throttled at 1/2
            # rate). These depend only on ones_c, so they run immediately.
            warm_ps = ps_stage.tile([P, QC], F32, tag="ps1", name="warm_ps")
            for _ in range(80):
                nc.tensor.matmul(
                    warm_ps[0:2, 0:2], ones_c, ones_c, start=True, stop=True
                )

            # Persistent per-core operands (feature-major xT/QT/KT, natural V).
            xT = big.tile([P, DT, S], sb_dt, tag="xT", name="xT")
            qT = big.tile([P, DT, S], sb_dt, tag="qT", name="qT")
            kT = big.tile([P, DT, S], sb_dt, tag="kT", name="kT")
            # +2 ones columns: fp32r matmuls need even free sizes, so the
            # denominator column is duplicated (col D and D+1 both = 1.0)
            vA = big.tile([P, ST, D + 2], sb_dt, tag="vA", name="vA")
            wqT = big.tile([P, DT, D], sb_dt, tag="wqT", name="wqT")
            wkT = big.tile([P, DT, D], sb_dt, tag="wkT", name="wkT")
            wvT = big.tile([P, DT, D], sb_dt, tag="wvT", name="wvT")

            # ---- load pre-transposed operands -----------------------------
            if DIRECT:
                # one sync-queue DMA stream, ordered exactly as the PE
                # consumes: wv, x cols 0:512, wq, wk, x cols 512:2048
                def dma_w(w_dram, wT):
                    for dt_ in range(DT):
                        nc.sync.dma_start(
                            out=wT[:, dt_, :],
                            in_=w_dram[dt_ * P:(dt_ + 1) * P, :],
                        )

                def dma_x(qc):
                    for dt_ in range(DT):
                        nc.sync.dma_start(
                            out=xT[:, dt_, qc * QC:(qc + 1) * QC],
                            in_=xt[dt_ * P:(dt_ + 1) * P, qc * QC:(qc + 1) * QC],
                        )

                dma_w(wvt, wvT)
                dma_x(0)
                dma_w(wqt, wqT)
                dma_w(wkt, wkT)
                for qc in range(1, NQ):
                    dma_x(qc)
            else:
                for dt_ in range(DT):
                    for qc in range(NQ):
                        sx = stage_pool.tile([P, QC], F32, tag="sx", name="sx")
                        nc.sync.dma_start(
                            out=sx,
                            in_=xt[dt_ * P:(dt_ + 1) * P, qc * QC:(qc + 1) * QC],
                        )
                        nc.vector.tensor_copy(
                            xT[:, dt_, qc * QC:(qc + 1) * QC], sx
                        )
                for w_dram, wT in ((wvt, wvT), (wqt, wqT), (wkt, wkT)):
                    for dt_ in range(DT):
                        sw = stage_pool.tile([P, D], F32, tag="sw", name="sw")
                        nc.gpsimd.dma_start(
                            out=sw, in_=w_dram[dt_ * P:(dt_ + 1) * P, :]
                        )
                        nc.vector.tensor_copy(wT[:, dt_, :], sw)

            # ---- projections ---------------------------------------------
            def project_v(st):
                # V natural: V[s, e] = sum_d xT[d, s] * WvT[d, e]
                pv = ps_stage.tile([P, QC], F32, tag="ps1", name="pv")
                for dt_ in range(DT):
                    nc.tensor.matmul(
                        pv[:, 0:D],
                        xT[:, dt_, st * P:(st + 1) * P],
                        wvT[:, dt_, :],
                        start=(dt_ == 0),
                        stop=(dt_ == DT - 1),
                    )
                nc.vector.tensor_copy(vA[:, st, 0:D], pv[:, 0:D])
                nc.vector.tensor_copy(vA[:, st, D:D + 2], ones_c)

            # QT/KT feature-major: QT[e, s] = sum_d WqT[d, e] * xT[d, s]
            def project_qk_chunk(wT, dst, qc, et):
                pp = ps_stage.tile([P, QC], F32, tag="ps1", name="pp")
                for dt_ in range(DT):
                    nc.tensor.matmul(
                        pp,
                        wT[:, dt_, et * P:(et + 1) * P],
                        xT[:, dt_, qc * QC:(qc + 1) * QC],
                        start=(dt_ == 0),
                        stop=(dt_ == DT - 1),
                    )
                nc.vector.tensor_copy(dst[:, et, qc * QC:(qc + 1) * QC], pp)

            def project_qk(wT, dst, qc):
                for et in range(DT):
                    project_qk_chunk(wT, dst, qc, et)

            # Upfront only what attention quarter 0 needs: all of K, plus Q's
            # first quarter. V and the later Q quarters are folded into the
            # attention loop below — there the PE dominates and the DVE
            # (which paces the PSUM->SBUF casts) has slack.
            for qc in range(NQ):
                project_qk(wkT, kT, qc)
            project_qk(wqT, qT, 0)

            # ---- attention, one 512-wide q chunk at a time ----------------
            for c in range(NQ):
                accs = [
                    ps_acc.tile([P, D + 2], F32, tag="acc", name="acc")
                    for _ in range(4)
                ]

                def emit_pv(kt_i, ex):
                    for qs in range(4):
                        nc.tensor.matmul(
                            accs[qs],
                            ex[:, qs * P:(qs + 1) * P],
                            vA[:, kt_i, :],
                            start=(kt_i == 0),
                            stop=(kt_i == ST - 1),
                        )

                pending = None
                for kt_i in range(ST):
                    if c == 0:
                        # produce vA[kt_i] here; the PV consumer for it runs
                        # one iteration later (software pipelined)
                        project_v(kt_i)
                    if c < NQ - 1:
                        # stream next quarter's Q projection through the loop
                        if kt_i == 1:
                            project_qk_chunk(wqT, qT, c + 1, 0)
                        elif kt_i == 6:
                            project_qk_chunk(wqT, qT, c + 1, 1)
                        elif kt_i == 11:
                            project_qk_chunk(wqT, qT, c + 1, 2)
                    pa = ps_stage.tile([P, QC], F32, tag="ps1", name="pa")
                    for et in range(DT):
                        nc.tensor.matmul(
                            pa,
                            kT[:, et, kt_i * P:(kt_i + 1) * P],
                            qT[:, et, c * QC:(c + 1) * QC],
                            start=(et == 0),
                            stop=(et == DT - 1),
                        )
                    ex = ex_pool.tile([P, QC], sb_dt, tag="ex", name="ex")
                    nc.scalar.activation(
                        ex, pa, mybir.ActivationFunctionType.Exp
                    )
                    # software-pipeline PV by one k-tile so PE never waits
                    # on the exp that was just issued
                    if pending is not None:
                        emit_pv(*pending)
                    pending = (kt_i, ex)
                emit_pv(*pending)

                for qs in range(4):
                    rec = small_pool.tile([P, 1], F32, tag="rec", name="rec")
                    nc.vector.reciprocal(rec, accs[qs][:, D:D + 1])
                    ob = ob_pool.tile([P, D], F32, tag="ob", name="ob")
                    nc.vector.tensor_scalar_mul(ob, accs[qs][:, 0:D], rec)
                    qt_row = (c * 4 + qs) * P
                    nc.sync.dma_start(out=out[qt_row:qt_row + P, :], in_=ob)

    nc.compile()
    return nc


_NC = None
_FAST = None


def _get_nc():
    global _NC
    if _NC is None:
        _NC = _build()
    return _NC


def _fast_runner():
    """Build (once) a jitted shard_map callable over the 8 cores.

    Mirrors bass2jax.run_bass_via_pjrt's multi-core branch, but keeps the
    jitted function alive across kernel() calls so repeat invocations skip
    re-trace/re-compile.
    """
    global _FAST
    if _FAST is not None:
        return _FAST
    import jax
    from jax.experimental.shard_map import shard_map
    from jax.sharding import Mesh, PartitionSpec

    from concourse import bass2jax

    nc = _get_nc()
    bass2jax.install_neuronx_cc_hook()

    in_names = ["xt", "wqt", "wkt", "wvt"]
    out_aval = jax.core.ShapedArray((S, D), np.float32)

    def _body(*args):
        operands = list(args)
        operands.append(bass2jax.partition_id_tensor())
        outs = bass2jax._bass_exec_p.bind(
            *operands,
            out_avals=(out_aval,),
            in_names=tuple(in_names) + ("out", "partition_id"),
            out_names=("out",),
            lowering_input_output_aliases=(),
            sim_require_finite=True,
            sim_require_nnan=True,
            nc=nc,
        )
        return tuple(outs)

    devices = jax.devices()[:NB]
    mesh = Mesh(np.asarray(devices), ("core",))
    n_in = len(in_names) + 1  # + donated zero output
    fn = jax.jit(
        shard_map(
            _body,
            mesh=mesh,
            in_specs=(PartitionSpec("core"),) * n_in,
            out_specs=(PartitionSpec("core"),),
            check_rep=False,
        ),
        donate_argnums=(n_in - 1,),
        keep_unused=True,
    )
    _FAST = fn
    return fn


def _marshal(att_input, Wq, Wk, Wv):
    att_input = np.asarray(att_input, dtype=np.float32)
    # pre-transposed per-core x and shared weights (layout only, no FLOPs)
    xts = np.ascontiguousarray(att_input.transpose(0, 2, 1))  # [NB, D, S]
    wts = [
        np.ascontiguousarray(np.asarray(w, dtype=np.float32).T)
        for w in (Wq, Wk, Wv)
    ]
    return xts, wts


def run(att_input, Wq, Wk, Wv, trace=False):
    xts, wts = _marshal(att_input, Wq, Wk, Wv)
    if trace:
        in_maps = [
            {"xt": xts[b], "wqt": wts[0], "wkt": wts[1], "wvt": wts[2]}
            for b in range(NB)
        ]
        res = bass_utils.run_bass_kernel_spmd(
            _get_nc(), in_maps, core_ids=list(range(NB)), trace=True
        )
        out = np.stack([res.results[b]["out"] for b in range(NB)], axis=0)
        return out.astype(np.float32, copy=False), res

    fn = _fast_runner()
    xs = xts.reshape(NB * D, S)
    ws = [np.concatenate([w] * NB, axis=0) for w in wts]
    zeros = np.zeros((NB * S, D), np.float32)
    (out,) = fn(xs, *ws, zeros)
    return np.asarray(out).reshape(NB, S, D).astype(np.float32, copy=False), None


def kernel(att_input, Wq, Wk, Wv):
    out, _ = run(att_input, Wq, Wk, Wv)
    return out


# revision 23
# speedup vs baseline: 1.0085x; 1.0085x over previous
"""Single-head attention (B=8, S=2048, D=384) on 8 NeuronCores.

Sharding: data-parallel over batch — core b computes batch element b
entirely (QKV projections + softmax(Q K^T) V), weights replicated.

Host-side marshalling (part of kernel()'s sharding step): x is fed
pre-transposed per core as xT [D, S], and the weights pre-transposed as
WqT/WkT/WvT [D, D] (zero host FLOPs — layout only). This removes every
PE-transpose from the device kernel.

Per-core dataflow (all on one NeuronCore, f32 in/out):
  - QT = (WqT)^T-blocks @ xT and KT likewise (both [D, S], feature-major),
    V = x @ Wv^T in natural [S, D] layout — all from xT via the PE, with a
    ones column pair appended to V -> vA [S, D+2].
  - scores^T tile alphaT[k, q] = KT-block^T @ QT-block accumulated over the
    3 e-tiles; exp() on ScalarE (no max subtraction needed: logits are
    ~N(0, 42), |logit| < ~45 << 88, so fp32 exp cannot overflow; softmax is
    shift-invariant so the result matches the reference's max-subtracted
    computation up to rounding).
  - out_raw[q, :D] and the softmax denominator accumulate TOGETHER via
    out_acc[q, 0:D+2] += expT[k, q-block]^T @ vA[k-block, :]  (the ones
    columns of vA make column D equal sum_k exp) — no cross-partition
    reduction ever needed.
  - out[q, e] = out_raw[q, e] * (1 / out_acc[q, D]).

Matmuls run as float32r (full PE rate at N>=256); fp32 PSUM accumulation.
"""

import os
import numpy as np

import concourse.bass as bass
import concourse.bacc as bacc
import concourse.tile as tile
from concourse import mybir
from concourse import bass_utils

P = 128          # partitions / PE tile edge
S = 2048         # sequence length per core
D = 384          # model dim
NB = 8           # batch == number of cores
DT = D // P      # 3 feature tiles
ST = S // P      # 16 sequence tiles
QC = 512         # q-column chunk (PSUM bank of f32)
NQ = S // QC     # 4 q chunks
F32 = mybir.dt.float32
F32R = mybir.dt.float32r
BF16 = mybir.dt.bfloat16

# "f32r" (default), "f32", or "bf16" — matmul operand precision.
MM_MODE = os.environ.get("ATT_MM_MODE", "f32r")
# 1: DMA straight into f32r operand tiles; 0: DMA to f32 staging + DVE cast
DIRECT = os.environ.get("ATT_DIRECT", "1") == "1"


def _build():
    sb_dt = {"f32r": F32R, "bf16": BF16, "f32": F32}[MM_MODE]

    nc = bacc.Bacc(
        "TRN2", target_bir_lowering=False, debug=False, enable_asserts=False
    )
    # DRAM inputs carry the matmul dtype so the direct DMA is cast-free
    # (float32r has identical 4-byte layout; numpy side stays float32)
    in_dt = {"f32r": F32R, "bf16": F32, "f32": F32}[MM_MODE] if DIRECT else F32
    xt = nc.dram_tensor("xt", [D, S], in_dt, kind="ExternalInput").ap()
    wqt = nc.dram_tensor("wqt", [D, D], in_dt, kind="ExternalInput").ap()
    wkt = nc.dram_tensor("wkt", [D, D], in_dt, kind="ExternalInput").ap()
    wvt = nc.dram_tensor("wvt", [D, D], in_dt, kind="ExternalInput").ap()
    out = nc.dram_tensor("out", [S, D], F32, kind="ExternalOutput").ap()

    with tile.TileContext(nc) as tc:
        with (
            tc.tile_pool(name="const", bufs=1) as const_pool,
            tc.tile_pool(name="big", bufs=1) as big,
            tc.tile_pool(name="stage", bufs=4) as stage_pool,
            tc.tile_pool(name="expool", bufs=3) as ex_pool,
            tc.tile_pool(name="obpool", bufs=3) as ob_pool,
            tc.tile_pool(name="smalls", bufs=4) as small_pool,
            tc.tile_pool(name="ps_stage", bufs=4, space="PSUM") as ps_stage,
            tc.tile_pool(name="ps_acc", bufs=4, space="PSUM") as ps_acc,
        ):
            ones_c = const_pool.tile([P, 2], F32, tag="ones", name="ones_c")
            nc.vector.memset(ones_c, 1.0)

            # HAM warmup: keep the PE busy during the DMA head so the clock
            # gate is at 8/8 when the first real matmul issues (~3.4us of
            # sustained activity flips it; idle PE starts throttled at 1/2
            # rate). These depend only on ones_c, so they run immediately.
            warm_ps = ps_stage.tile([P, QC], F32, tag="ps1", name="warm_ps")
            for _ in range(80):
                nc.tensor.matmul(
                    warm_ps[0:2, 0:2], ones_c, ones_c, start=True, stop=True
                )

            # Persistent per-core operands (feature-major xT/QT/KT, natural V).
            xT = big.tile([P, DT, S], sb_dt, tag="xT", name="xT")
            qT = big.tile([P, DT, S], sb_dt, tag="qT", name="qT")
            kT = big.tile([P, DT, S], sb_dt, tag="kT", name="kT")
            # +2 ones columns: fp32r matmuls need even free sizes, so the
            # denominator column is duplicated (col D and D+1 both = 1.0)
            vA = big.tile([P, ST, D + 2], sb_dt, tag="vA", name="vA")
            wqT = big.tile([P, DT, D], sb_dt, tag="wqT", name="wqT")
            wkT = big.tile([P, DT, D], sb_dt, tag="wkT", name="wkT")
            wvT = big.tile([P, DT, D], sb_dt, tag="wvT", name="wvT")

            # ---- load pre-transposed operands -----------------------------
            if DIRECT:
                # one sync-queue DMA stream, ordered exactly as the PE
                # consumes: wv, x cols 0:512, wq, wk, x cols 512:2048
                def dma_w(w_dram, wT):
                    for dt_ in range(DT):
                        nc.sync.dma_start(
                            out=wT[:, dt_, :],
                            in_=w_dram[dt_ * P:(dt_ + 1) * P, :],
                        )

                def dma_x(qc):
                    for dt_ in range(DT):
                        nc.sync.dma_start(
                            out=xT[:, dt_, qc * QC:(qc + 1) * QC],
                            in_=xt[dt_ * P:(dt_ + 1) * P, qc * QC:(qc + 1) * QC],
                        )

                # consumption order: K-projs (all x), Q qc0, V inside attn
                dma_w(wkt, wkT)
                dma_x(0)
                dma_x(1)
                dma_w(wqt, wqT)
                dma_x(2)
                dma_x(3)
                dma_w(wvt, wvT)
            else:
                for dt_ in range(DT):
                    for qc in range(NQ):
                        sx = stage_pool.tile([P, QC], F32, tag="sx", name="sx")
                        nc.sync.dma_start(
                            out=sx,
                            in_=xt[dt_ * P:(dt_ + 1) * P, qc * QC:(qc + 1) * QC],
                        )
                        nc.vector.tensor_copy(
                            xT[:, dt_, qc * QC:(qc + 1) * QC], sx
                        )
                for w_dram, wT in ((wvt, wvT), (wqt, wqT), (wkt, wkT)):
                    for dt_ in range(DT):
                        sw = stage_pool.tile([P, D], F32, tag="sw", name="sw")
                        nc.gpsimd.dma_start(
                            out=sw, in_=w_dram[dt_ * P:(dt_ + 1) * P, :]
                        )
                        nc.vector.tensor_copy(wT[:, dt_, :], sw)

            # ---- projections ---------------------------------------------
            def project_v(st):
                # V natural: V[s, e] = sum_d xT[d, s] * WvT[d, e]
                pv = ps_stage.tile([P, QC], F32, tag="ps1", name="pv")
                for dt_ in range(DT):
                    nc.tensor.matmul(
                        pv[:, 0:D],
                        xT[:, dt_, st * P:(st + 1) * P],
                        wvT[:, dt_, :],
                        start=(dt_ == 0),
                        stop=(dt_ == DT - 1),
                    )
                nc.vector.tensor_copy(vA[:, st, 0:D], pv[:, 0:D])
                nc.vector.tensor_copy(vA[:, st, D:D + 2], ones_c)

            # QT/KT feature-major: QT[e, s] = sum_d WqT[d, e] * xT[d, s]
            def project_qk_chunk(wT, dst, qc, et):
                pp = ps_stage.tile([P, QC], F32, tag="ps1", name="pp")
                for dt_ in range(DT):
                    nc.tensor.matmul(
                        pp,
                        wT[:, dt_, et * P:(et + 1) * P],
                        xT[:, dt_, qc * QC:(qc + 1) * QC],
                        start=(dt_ == 0),
                        stop=(dt_ == DT - 1),
                    )
                nc.vector.tensor_copy(dst[:, et, qc * QC:(qc + 1) * QC], pp)

            def project_qk(wT, dst, qc):
                for et in range(DT):
                    project_qk_chunk(wT, dst, qc, et)

            # Upfront only what attention quarter 0 needs: all of K, plus Q's
            # first quarter. V and the later Q quarters are folded into the
            # attention loop below — there the PE dominates and the DVE
            # (which paces the PSUM->SBUF casts) has slack.
            for qc in range(NQ):
                project_qk(wkT, kT, qc)
            project_qk(wqT, qT, 0)

            # ---- attention, one 512-wide q chunk at a time ----------------
            for c in range(NQ):
                accs = [
                    ps_acc.tile([P, D + 2], F32, tag="acc", name="acc")
                    for _ in range(4)
                ]

                def emit_pv(kt_i, ex):
                    for qs in range(4):
                        nc.tensor.matmul(
                            accs[qs],
                            ex[:, qs * P:(qs + 1) * P],
                            vA[:, kt_i, :],
                            start=(kt_i == 0),
                            stop=(kt_i == ST - 1),
                        )

                pending = None
                for kt_i in range(ST):
                    if c == 0:
                        # produce vA[kt_i] here; the PV consumer for it runs
                        # one iteration later (software pipelined)
                        project_v(kt_i)
                    if c < NQ - 1:
                        # stream next quarter's Q projection through the loop
                        if kt_i == 1:
                            project_qk_chunk(wqT, qT, c + 1, 0)
                        elif kt_i == 6:
                            project_qk_chunk(wqT, qT, c + 1, 1)
                        elif kt_i == 11:
                            project_qk_chunk(wqT, qT, c + 1, 2)
                    pa = ps_stage.tile([P, QC], F32, tag="ps1", name="pa")
                    for et in range(DT):
                        nc.tensor.matmul(
                            pa,
                            kT[:, et, kt_i * P:(kt_i + 1) * P],
                            qT[:, et, c * QC:(c + 1) * QC],
                            start=(et == 0),
                            stop=(et == DT - 1),
                        )
                    ex = ex_pool.tile([P, QC], sb_dt, tag="ex", name="ex")
                    nc.scalar.activation(
                        ex, pa, mybir.ActivationFunctionType.Exp
                    )
                    # software-pipeline PV by one k-tile so PE never waits
                    # on the exp that was just issued
                    if pending is not None:
                        emit_pv(*pending)
                    pending = (kt_i, ex)
                emit_pv(*pending)

                for qs in range(4):
                    rec = small_pool.tile([P, 1], F32, tag="rec", name="rec")
                    nc.vector.reciprocal(rec, accs[qs][:, D:D + 1])
                    ob = ob_pool.tile([P, D], F32, tag="ob", name="ob")
                    nc.vector.tensor_scalar_mul(ob, accs[qs][:, 0:D], rec)
                    qt_row = (c * 4 + qs) * P
                    nc.sync.dma_start(out=out[qt_row:qt_row + P, :], in_=ob)

    nc.compile()
    return nc


_NC = None
_FAST = None


def _get_nc():
    global _NC
    if _NC is None:
        _NC = _build()
    return _NC


def _fast_runner():
    """Build (once) a jitted shard_map callable over the 8 cores.

    Mirrors bass2jax.run_bass_via_pjrt's multi-core branch, but keeps the
    jitted function alive across kernel() calls so repeat invocations skip
    re-trace/re-compile.
    """
    global _FAST
    if _FAST is not None:
        return _FAST
    import jax
    from jax.experimental.shard_map import shard_map
    from jax.sharding import Mesh, PartitionSpec

    from concourse import bass2jax

    nc = _get_nc()
    bass2jax.install_neuronx_cc_hook()

    in_names = ["xt", "wqt", "wkt", "wvt"]
    out_aval = jax.core.ShapedArray((S, D), np.float32)

    def _body(*args):
        operands = list(args)
        operands.append(bass2jax.partition_id_tensor())
        outs = bass2jax._bass_exec_p.bind(
            *operands,
            out_avals=(out_aval,),
            in_names=tuple(in_names) + ("out", "partition_id"),
            out_names=("out",),
            lowering_input_output_aliases=(),
            sim_require_finite=True,
            sim_require_nnan=True,
            nc=nc,
        )
        return tuple(outs)

    devices = jax.devices()[:NB]
    mesh = Mesh(np.asarray(devices), ("core",))
    n_in = len(in_names) + 1  # + donated zero output
    fn = jax.jit(
        shard_map(
            _body,
            mesh=mesh,
            in_specs=(PartitionSpec("core"),) * n_in,
            out_specs=(PartitionSpec("core"),),
            check_rep=False,
        ),
        donate_argnums=(n_in - 1,),
        keep_unused=True,
    )
    _FAST = fn
    return fn


def _marshal(att_input, Wq, Wk, Wv):
    att_input = np.asarray(att_input, dtype=np.float32)
    # pre-transposed per-core x and shared weights (layout only, no FLOPs)
    xts = np.ascontiguousarray(att_input.transpose(0, 2, 1))  # [NB, D, S]
    wts = [
        np.ascontiguousarray(np.asarray(w, dtype=np.float32).T)
        for w in (Wq, Wk, Wv)
    ]
    return xts, wts


def run(att_input, Wq, Wk, Wv, trace=False):
    xts, wts = _marshal(att_input, Wq, Wk, Wv)
    if trace:
        in_maps = [
            {"xt": xts[b], "wqt": wts[0], "wkt": wts[1], "wvt": wts[2]}
            for b in range(NB)
        ]
        res = bass_utils.run_bass_kernel_spmd(
            _get_nc(), in_maps, core_ids=list(range(NB)), trace=True
        )
        out = np.stack([res.results[b]["out"] for b in range(NB)], axis=0)
        return out.astype(np.float32, copy=False), res

    fn = _fast_runner()
    xs = xts.reshape(NB * D, S)
    ws = [np.concatenate([w] * NB, axis=0) for w in wts]
    zeros = np.zeros((NB * S, D), np.float32)
    (out,) = fn(xs, *ws, zeros)
    return np.asarray(out).reshape(NB, S, D).astype(np.float32, copy=False), None


def kernel(att_input, Wq, Wk, Wv):
    out, _ = run(att_input, Wq, Wk, Wv)
    return out


# revision 24
# speedup vs baseline: 1.0194x; 1.0108x over previous
"""Single-head attention (B=8, S=2048, D=384) on 8 NeuronCores.

Sharding: data-parallel over batch — core b computes batch element b
entirely (QKV projections + softmax(Q K^T) V), weights replicated.

Host-side marshalling (part of kernel()'s sharding step): x is fed
pre-transposed per core as xT [D, S], and the weights pre-transposed as
WqT/WkT/WvT [D, D] (zero host FLOPs — layout only). This removes every
PE-transpose from the device kernel.

Per-core dataflow (all on one NeuronCore, f32 in/out):
  - QT = (WqT)^T-blocks @ xT and KT likewise (both [D, S], feature-major),
    V = x @ Wv^T in natural [S, D] layout — all from xT via the PE, with a
    ones column pair appended to V -> vA [S, D+2].
  - scores^T tile alphaT[k, q] = KT-block^T @ QT-block accumulated over the
    3 e-tiles; exp() on ScalarE (no max subtraction needed: logits are
    ~N(0, 42), |logit| < ~45 << 88, so fp32 exp cannot overflow; softmax is
    shift-invariant so the result matches the reference's max-subtracted
    computation up to rounding).
  - out_raw[q, :D] and the softmax denominator accumulate TOGETHER via
    out_acc[q, 0:D+2] += expT[k, q-block]^T @ vA[k-block, :]  (the ones
    columns of vA make column D equal sum_k exp) — no cross-partition
    reduction ever needed.
  - out[q, e] = out_raw[q, e] * (1 / out_acc[q, D]).

Matmuls run as float32r (full PE rate at N>=256); fp32 PSUM accumulation.
"""

import os
import numpy as np

import concourse.bass as bass
import concourse.bacc as bacc
import concourse.tile as tile
from concourse import mybir
from concourse import bass_utils

P = 128          # partitions / PE tile edge
S = 2048         # sequence length per core
D = 384          # model dim
NB = 8           # batch == number of cores
DT = D // P      # 3 feature tiles
ST = S // P      # 16 sequence tiles
QC = 512         # q-column chunk (PSUM bank of f32)
NQ = S // QC     # 4 q chunks
F32 = mybir.dt.float32
F32R = mybir.dt.float32r
BF16 = mybir.dt.bfloat16

# "f32r" (default), "f32", or "bf16" — matmul operand precision.
MM_MODE = os.environ.get("ATT_MM_MODE", "f32r")
# 1: DMA straight into f32r operand tiles; 0: DMA to f32 staging + DVE cast
DIRECT = os.environ.get("ATT_DIRECT", "1") == "1"


def _build():
    sb_dt = {"f32r": F32R, "bf16": BF16, "f32": F32}[MM_MODE]

    nc = bacc.Bacc(
        "TRN2", target_bir_lowering=False, debug=False, enable_asserts=False
    )
    # DRAM inputs carry the matmul dtype so the direct DMA is cast-free
    # (float32r has identical 4-byte layout; numpy side stays float32)
    in_dt = {"f32r": F32R, "bf16": F32, "f32": F32}[MM_MODE] if DIRECT else F32
    xt = nc.dram_tensor("xt", [D, S], in_dt, kind="ExternalInput").ap()
    wqt = nc.dram_tensor("wqt", [D, D], in_dt, kind="ExternalInput").ap()
    wkt = nc.dram_tensor("wkt", [D, D], in_dt, kind="ExternalInput").ap()
    wvt = nc.dram_tensor("wvt", [D, D], in_dt, kind="ExternalInput").ap()
    out = nc.dram_tensor("out", [S, D], F32, kind="ExternalOutput").ap()

    with tile.TileContext(nc) as tc:
        with (
            tc.tile_pool(name="const", bufs=1) as const_pool,
            tc.tile_pool(name="big", bufs=1) as big,
            tc.tile_pool(name="stage", bufs=4) as stage_pool,
            tc.tile_pool(name="expool", bufs=3) as ex_pool,
            tc.tile_pool(name="obpool", bufs=3) as ob_pool,
            tc.tile_pool(name="smalls", bufs=4) as small_pool,
            tc.tile_pool(name="ps_stage", bufs=4, space="PSUM") as ps_stage,
            tc.tile_pool(name="ps_acc", bufs=4, space="PSUM") as ps_acc,
        ):
            ones_c = const_pool.tile([P, 2], F32, tag="ones", name="ones_c")
            nc.vector.memset(ones_c, 1.0)


            # Persistent per-core operands (feature-major xT/QT/KT, natural V).
            xT = big.tile([P, DT, S], sb_dt, tag="xT", name="xT")
            qT = big.tile([P, DT, S], sb_dt, tag="qT", name="qT")
            kT = big.tile([P, DT, S], sb_dt, tag="kT", name="kT")
            # +2 ones columns: fp32r matmuls need even free sizes, so the
            # denominator column is duplicated (col D and D+1 both = 1.0)
            vA = big.tile([P, ST, D + 2], sb_dt, tag="vA", name="vA")
            wqT = big.tile([P, DT, D], sb_dt, tag="wqT", name="wqT")
            wkT = big.tile([P, DT, D], sb_dt, tag="wkT", name="wkT")
            wvT = big.tile([P, DT, D], sb_dt, tag="wvT", name="wvT")

            # ---- load pre-transposed operands -----------------------------
            if DIRECT:
                # one sync-queue DMA stream, ordered exactly as the PE
                # consumes: wv, x cols 0:512, wq, wk, x cols 512:2048
                def dma_w(w_dram, wT):
                    for dt_ in range(DT):
                        nc.sync.dma_start(
                            out=wT[:, dt_, :],
                            in_=w_dram[dt_ * P:(dt_ + 1) * P, :],
                        )

                def dma_x(qc):
                    for dt_ in range(DT):
                        nc.sync.dma_start(
                            out=xT[:, dt_, qc * QC:(qc + 1) * QC],
                            in_=xt[dt_ * P:(dt_ + 1) * P, qc * QC:(qc + 1) * QC],
                        )

                dma_w(wvt, wvT)
                dma_x(0)
                dma_w(wqt, wqT)
                dma_w(wkt, wkT)
                for qc in range(1, NQ):
                    dma_x(qc)
            else:
                for dt_ in range(DT):
                    for qc in range(NQ):
                        sx = stage_pool.tile([P, QC], F32, tag="sx", name="sx")
                        nc.sync.dma_start(
                            out=sx,
                            in_=xt[dt_ * P:(dt_ + 1) * P, qc * QC:(qc + 1) * QC],
                        )
                        nc.vector.tensor_copy(
                            xT[:, dt_, qc * QC:(qc + 1) * QC], sx
                        )
                for w_dram, wT in ((wvt, wvT), (wqt, wqT), (wkt, wkT)):
                    for dt_ in range(DT):
                        sw = stage_pool.tile([P, D], F32, tag="sw", name="sw")
                        nc.gpsimd.dma_start(
                            out=sw, in_=w_dram[dt_ * P:(dt_ + 1) * P, :]
                        )
                        nc.vector.tensor_copy(wT[:, dt_, :], sw)

            # ---- projections ---------------------------------------------
            def project_v(st):
                # V natural: V[s, e] = sum_d xT[d, s] * WvT[d, e]
                pv = ps_stage.tile([P, QC], F32, tag="ps1", name="pv")
                for dt_ in range(DT):
                    nc.tensor.matmul(
                        pv[:, 0:D],
                        xT[:, dt_, st * P:(st + 1) * P],
                        wvT[:, dt_, :],
                        start=(dt_ == 0),
                        stop=(dt_ == DT - 1),
                    )
                nc.vector.tensor_copy(vA[:, st, 0:D], pv[:, 0:D])
                nc.vector.tensor_copy(vA[:, st, D:D + 2], ones_c)

            # QT/KT feature-major: QT[e, s] = sum_d WqT[d, e] * xT[d, s]
            def project_qk_chunk(wT, dst, qc, et):
                pp = ps_stage.tile([P, QC], F32, tag="ps1", name="pp")
                for dt_ in range(DT):
                    nc.tensor.matmul(
                        pp,
                        wT[:, dt_, et * P:(et + 1) * P],
                        xT[:, dt_, qc * QC:(qc + 1) * QC],
                        start=(dt_ == 0),
                        stop=(dt_ == DT - 1),
                    )
                nc.vector.tensor_copy(dst[:, et, qc * QC:(qc + 1) * QC], pp)

            def project_qk(wT, dst, qc):
                for et in range(DT):
                    project_qk_chunk(wT, dst, qc, et)

            # per 512-col x chunk: V rows, then K/Q columns — matches the
            # DMA arrival order so the PE never waits past the first chunk
            for qc in range(NQ):
                for st in range(qc * 4, qc * 4 + 4):
                    project_v(st)
                project_qk(wkT, kT, qc)
                project_qk(wqT, qT, qc)

            # ---- attention, one 512-wide q chunk at a time ----------------
            for c in range(NQ):
                accs = [
                    ps_acc.tile([P, D + 2], F32, tag="acc", name="acc")
                    for _ in range(4)
                ]

                def emit_pv(kt_i, ex):
                    for qs in range(4):
                        nc.tensor.matmul(
                            accs[qs],
                            ex[:, qs * P:(qs + 1) * P],
                            vA[:, kt_i, :],
                            start=(kt_i == 0),
                            stop=(kt_i == ST - 1),
                        )

                pending = None
                for kt_i in range(ST):
                    pa = ps_stage.tile([P, QC], F32, tag="ps1", name="pa")
                    for et in range(DT):
                        nc.tensor.matmul(
                            pa,
                            kT[:, et, kt_i * P:(kt_i + 1) * P],
                            qT[:, et, c * QC:(c + 1) * QC],
                            start=(et == 0),
                            stop=(et == DT - 1),
                        )
                    ex = ex_pool.tile([P, QC], sb_dt, tag="ex", name="ex")
                    nc.scalar.activation(
                        ex, pa, mybir.ActivationFunctionType.Exp
                    )
                    # software-pipeline PV by one k-tile so PE never waits
                    # on the exp that was just issued
                    if pending is not None:
                        emit_pv(*pending)
                    pending = (kt_i, ex)
                emit_pv(*pending)

                for qs in range(4):
                    rec = small_pool.tile([P, 1], F32, tag="rec", name="rec")
                    nc.vector.reciprocal(rec, accs[qs][:, D:D + 1])
                    ob = ob_pool.tile([P, D], F32, tag="ob", name="ob")
                    nc.vector.tensor_scalar_mul(ob, accs[qs][:, 0:D], rec)
                    qt_row = (c * 4 + qs) * P
                    nc.sync.dma_start(out=out[qt_row:qt_row + P, :], in_=ob)

    nc.compile()
    return nc


_NC = None
_FAST = None


def _get_nc():
    global _NC
    if _NC is None:
        _NC = _build()
    return _NC


def _fast_runner():
    """Build (once) a jitted shard_map callable over the 8 cores.

    Mirrors bass2jax.run_bass_via_pjrt's multi-core branch, but keeps the
    jitted function alive across kernel() calls so repeat invocations skip
    re-trace/re-compile.
    """
    global _FAST
    if _FAST is not None:
        return _FAST
    import jax
    from jax.experimental.shard_map import shard_map
    from jax.sharding import Mesh, PartitionSpec

    from concourse import bass2jax

    nc = _get_nc()
    bass2jax.install_neuronx_cc_hook()

    in_names = ["xt", "wqt", "wkt", "wvt"]
    out_aval = jax.core.ShapedArray((S, D), np.float32)

    def _body(*args):
        operands = list(args)
        operands.append(bass2jax.partition_id_tensor())
        outs = bass2jax._bass_exec_p.bind(
            *operands,
            out_avals=(out_aval,),
            in_names=tuple(in_names) + ("out", "partition_id"),
            out_names=("out",),
            lowering_input_output_aliases=(),
            sim_require_finite=True,
            sim_require_nnan=True,
            nc=nc,
        )
        return tuple(outs)

    devices = jax.devices()[:NB]
    mesh = Mesh(np.asarray(devices), ("core",))
    n_in = len(in_names) + 1  # + donated zero output
    fn = jax.jit(
        shard_map(
            _body,
            mesh=mesh,
            in_specs=(PartitionSpec("core"),) * n_in,
            out_specs=(PartitionSpec("core"),),
            check_rep=False,
        ),
        donate_argnums=(n_in - 1,),
        keep_unused=True,
    )
    _FAST = fn
    return fn


def _marshal(att_input, Wq, Wk, Wv):
    att_input = np.asarray(att_input, dtype=np.float32)
    # pre-transposed per-core x and shared weights (layout only, no FLOPs)
    xts = np.ascontiguousarray(att_input.transpose(0, 2, 1))  # [NB, D, S]
    wts = [
        np.ascontiguousarray(np.asarray(w, dtype=np.float32).T)
        for w in (Wq, Wk, Wv)
    ]
    return xts, wts


def run(att_input, Wq, Wk, Wv, trace=False):
    xts, wts = _marshal(att_input, Wq, Wk, Wv)
    if trace:
        in_maps = [
            {"xt": xts[b], "wqt": wts[0], "wkt": wts[1], "wvt": wts[2]}
            for b in range(NB)
        ]
        res = bass_utils.run_bass_kernel_spmd(
            _get_nc(), in_maps, core_ids=list(range(NB)), trace=True
        )
        out = np.stack([res.results[b]["out"] for b in range(NB)], axis=0)
        return out.astype(np.float32, copy=False), res

    fn = _fast_runner()
    xs = xts.reshape(NB * D, S)
    ws = [np.concatenate([w] * NB, axis=0) for w in wts]
    zeros = np.zeros((NB * S, D), np.float32)
    (out,) = fn(xs, *ws, zeros)
    return np.asarray(out).reshape(NB, S, D).astype(np.float32, copy=False), None


def kernel(att_input, Wq, Wk, Wv):
    out, _ = run(att_input, Wq, Wk, Wv)
    return out


# revision 27
# speedup vs baseline: 1.0416x; 1.0218x over previous
"""Single-head attention (B=8, S=2048, D=384) on 8 NeuronCores.

Sharding: data-parallel over batch — core b computes batch element b
entirely (QKV projections + softmax(Q K^T) V), weights replicated.

Host-side marshalling (part of kernel()'s sharding step): x is fed
pre-transposed per core as xT [D, S], and the weights pre-transposed as
WqT/WkT/WvT [D, D] (zero host FLOPs — layout only). This removes every
PE-transpose from the device kernel.

Per-core dataflow (all on one NeuronCore, f32 in/out):
  - QT = (WqT)^T-blocks @ xT and KT likewise (both [D, S], feature-major),
    V = x @ Wv^T in natural [S, D] layout — all from xT via the PE, with a
    ones column pair appended to V -> vA [S, D+2].
  - scores^T tile alphaT[k, q] = KT-block^T @ QT-block accumulated over the
    3 e-tiles; exp() on ScalarE (no max subtraction needed: logits are
    ~N(0, 42), |logit| < ~45 << 88, so fp32 exp cannot overflow; softmax is
    shift-invariant so the result matches the reference's max-subtracted
    computation up to rounding).
  - out_raw[q, :D] and the softmax denominator accumulate TOGETHER via
    out_acc[q, 0:D+2] += expT[k, q-block]^T @ vA[k-block, :]  (the ones
    columns of vA make column D equal sum_k exp) — no cross-partition
    reduction ever needed.
  - out[q, e] = out_raw[q, e] * (1 / out_acc[q, D]).

Matmuls run as float32r (full PE rate at N>=256); fp32 PSUM accumulation.
"""

import os
import numpy as np

import concourse.bass as bass
import concourse.bacc as bacc
import concourse.tile as tile
from concourse import mybir
from concourse import bass_utils

P = 128          # partitions / PE tile edge
S = 2048         # sequence length per core
D = 384          # model dim
NB = 8           # batch == number of cores
DT = D // P      # 3 feature tiles
ST = S // P      # 16 sequence tiles
QC = 512         # q-column chunk (PSUM bank of f32)
NQ = S // QC     # 4 q chunks
F32 = mybir.dt.float32
F32R = mybir.dt.float32r
BF16 = mybir.dt.bfloat16

# "f32r" (default), "f32", or "bf16" — matmul operand precision.
MM_MODE = os.environ.get("ATT_MM_MODE", "f32r")
# 1: DMA straight into f32r operand tiles; 0: DMA to f32 staging + DVE cast
DIRECT = os.environ.get("ATT_DIRECT", "1") == "1"


def _build():
    sb_dt = {"f32r": F32R, "bf16": BF16, "f32": F32, "hybrid": F32R}[MM_MODE]
    # hybrid: QK path stays f32r, PV path (exp weights x V) runs bf16 —
    # exp rounding largely cancels between numerator and denominator
    pv_dt = BF16 if MM_MODE in ("bf16", "hybrid") else sb_dt

    nc = bacc.Bacc(
        "TRN2", target_bir_lowering=False, debug=False, enable_asserts=False
    )
    # DRAM inputs carry the matmul dtype so the direct DMA is cast-free
    # (float32r has identical 4-byte layout; bf16 is converted on host)
    in_dt = (
        {"f32r": F32R, "bf16": BF16, "f32": F32, "hybrid": F32R}[MM_MODE]
        if DIRECT
        else F32
    )
    xt = nc.dram_tensor("xt", [D, S], in_dt, kind="ExternalInput").ap()
    wqt = nc.dram_tensor("wqt", [D, D], in_dt, kind="ExternalInput").ap()
    wkt = nc.dram_tensor("wkt", [D, D], in_dt, kind="ExternalInput").ap()
    wvt = nc.dram_tensor("wvt", [D, D], in_dt, kind="ExternalInput").ap()
    out = nc.dram_tensor("out", [S, D], F32, kind="ExternalOutput").ap()

    with tile.TileContext(nc) as tc:
        with (
            tc.tile_pool(name="const", bufs=1) as const_pool,
            tc.tile_pool(name="big", bufs=1) as big,
            tc.tile_pool(name="stage", bufs=4) as stage_pool,
            tc.tile_pool(name="expool", bufs=3) as ex_pool,
            tc.tile_pool(name="obpool", bufs=3) as ob_pool,
            tc.tile_pool(name="smalls", bufs=4) as small_pool,
            tc.tile_pool(name="ps_stage", bufs=4, space="PSUM") as ps_stage,
            tc.tile_pool(name="ps_acc", bufs=4, space="PSUM") as ps_acc,
        ):
            ones_c = const_pool.tile([P, 2], F32, tag="ones", name="ones_c")
            nc.vector.memset(ones_c, 1.0)


            # Persistent per-core operands (feature-major xT/QT/KT, natural V).
            xT = big.tile([P, DT, S], sb_dt, tag="xT", name="xT")
            qT = big.tile([P, DT, S], sb_dt, tag="qT", name="qT")
            kT = big.tile([P, DT, S], sb_dt, tag="kT", name="kT")
            # +2 ones columns: fp32r matmuls need even free sizes, so the
            # denominator column is duplicated (col D and D+1 both = 1.0)
            vA = big.tile([P, ST, D + 2], pv_dt, tag="vA", name="vA")
            wqT = big.tile([P, DT, D], sb_dt, tag="wqT", name="wqT")
            wkT = big.tile([P, DT, D], sb_dt, tag="wkT", name="wkT")
            wvT = big.tile([P, DT, D], sb_dt, tag="wvT", name="wvT")

            # ---- load pre-transposed operands -----------------------------
            if DIRECT:
                # one sync-queue DMA stream, ordered exactly as the PE
                # consumes: wv, x cols 0:512, wq, wk, x cols 512:2048
                def dma_w(w_dram, wT):
                    for dt_ in range(DT):
                        nc.sync.dma_start(
                            out=wT[:, dt_, :],
                            in_=w_dram[dt_ * P:(dt_ + 1) * P, :],
                        )

                def dma_x(qc):
                    for dt_ in range(DT):
                        nc.sync.dma_start(
                            out=xT[:, dt_, qc * QC:(qc + 1) * QC],
                            in_=xt[dt_ * P:(dt_ + 1) * P, qc * QC:(qc + 1) * QC],
                        )

                dma_w(wvt, wvT)
                dma_x(0)
                dma_w(wqt, wqT)
                dma_w(wkt, wkT)
                for qc in range(1, NQ):
                    dma_x(qc)
            else:
                for dt_ in range(DT):
                    for qc in range(NQ):
                        sx = stage_pool.tile([P, QC], F32, tag="sx", name="sx")
                        nc.sync.dma_start(
                            out=sx,
                            in_=xt[dt_ * P:(dt_ + 1) * P, qc * QC:(qc + 1) * QC],
                        )
                        nc.vector.tensor_copy(
                            xT[:, dt_, qc * QC:(qc + 1) * QC], sx
                        )
                for w_dram, wT in ((wvt, wvT), (wqt, wqT), (wkt, wkT)):
                    for dt_ in range(DT):
                        sw = stage_pool.tile([P, D], F32, tag="sw", name="sw")
                        nc.gpsimd.dma_start(
                            out=sw, in_=w_dram[dt_ * P:(dt_ + 1) * P, :]
                        )
                        nc.vector.tensor_copy(wT[:, dt_, :], sw)

            # ---- projections ---------------------------------------------
            def project_v(st):
                # V natural: V[s, e] = sum_d xT[d, s] * WvT[d, e]
                pv = ps_stage.tile([P, QC], F32, tag="ps1", name="pv")
                for dt_ in range(DT):
                    nc.tensor.matmul(
                        pv[:, 0:D],
                        xT[:, dt_, st * P:(st + 1) * P],
                        wvT[:, dt_, :],
                        start=(dt_ == 0),
                        stop=(dt_ == DT - 1),
                    )
                nc.vector.tensor_copy(vA[:, st, 0:D], pv[:, 0:D])
                nc.vector.tensor_copy(vA[:, st, D:D + 2], ones_c)

            # QT/KT feature-major: QT[e, s] = sum_d WqT[d, e] * xT[d, s]
            def project_qk_chunk(wT, dst, qc, et):
                pp = ps_stage.tile([P, QC], F32, tag="ps1", name="pp")
                for dt_ in range(DT):
                    nc.tensor.matmul(
                        pp,
                        wT[:, dt_, et * P:(et + 1) * P],
                        xT[:, dt_, qc * QC:(qc + 1) * QC],
                        start=(dt_ == 0),
                        stop=(dt_ == DT - 1),
                    )
                nc.vector.tensor_copy(dst[:, et, qc * QC:(qc + 1) * QC], pp)

            def project_qk(wT, dst, qc):
                for et in range(DT):
                    project_qk_chunk(wT, dst, qc, et)

            # per 512-col x chunk: V rows, then K/Q columns — matches the
            # DMA arrival order so the PE never waits past the first chunk
            for qc in range(NQ):
                for st in range(qc * 4, qc * 4 + 4):
                    project_v(st)
                project_qk(wkT, kT, qc)
                project_qk(wqT, qT, qc)

            # ---- attention, one 512-wide q chunk at a time ----------------
            for c in range(NQ):
                accs = [
                    ps_acc.tile([P, D + 2], F32, tag="acc", name="acc")
                    for _ in range(4)
                ]

                def emit_pv(kt_i, ex):
                    for qs in range(4):
                        nc.tensor.matmul(
                            accs[qs],
                            ex[:, qs * P:(qs + 1) * P],
                            vA[:, kt_i, :],
                            start=(kt_i == 0),
                            stop=(kt_i == ST - 1),
                        )

                pending = None
                for kt_i in range(ST):
                    pa = ps_stage.tile([P, QC], F32, tag="ps1", name="pa")
                    for et in range(DT):
                        nc.tensor.matmul(
                            pa,
                            kT[:, et, kt_i * P:(kt_i + 1) * P],
                            qT[:, et, c * QC:(c + 1) * QC],
                            start=(et == 0),
                            stop=(et == DT - 1),
                        )
                    ex = ex_pool.tile([P, QC], pv_dt, tag="ex", name="ex")
                    nc.scalar.activation(
                        ex, pa, mybir.ActivationFunctionType.Exp
                    )
                    # software-pipeline PV by one k-tile so PE never waits
                    # on the exp that was just issued
                    if pending is not None:
                        emit_pv(*pending)
                    pending = (kt_i, ex)
                emit_pv(*pending)

                for qs in range(4):
                    rec = small_pool.tile([P, 1], F32, tag="rec", name="rec")
                    nc.vector.reciprocal(rec, accs[qs][:, D:D + 1])
                    ob = ob_pool.tile([P, D], F32, tag="ob", name="ob")
                    nc.vector.tensor_scalar_mul(ob, accs[qs][:, 0:D], rec)
                    qt_row = (c * 4 + qs) * P
                    nc.sync.dma_start(out=out[qt_row:qt_row + P, :], in_=ob)

    nc.compile()
    return nc


_NC = None
_FAST = None


def _get_nc():
    global _NC
    if _NC is None:
        _NC = _build()
    return _NC


def _fast_runner():
    """Build (once) a jitted shard_map callable over the 8 cores.

    Mirrors bass2jax.run_bass_via_pjrt's multi-core branch, but keeps the
    jitted function alive across kernel() calls so repeat invocations skip
    re-trace/re-compile.
    """
    global _FAST
    if _FAST is not None:
        return _FAST
    import jax
    from jax.experimental.shard_map import shard_map
    from jax.sharding import Mesh, PartitionSpec

    from concourse import bass2jax

    nc = _get_nc()
    bass2jax.install_neuronx_cc_hook()

    in_names = ["xt", "wqt", "wkt", "wvt"]
    out_aval = jax.core.ShapedArray((S, D), np.float32)

    def _body(*args):
        operands = list(args)
        operands.append(bass2jax.partition_id_tensor())
        outs = bass2jax._bass_exec_p.bind(
            *operands,
            out_avals=(out_aval,),
            in_names=tuple(in_names) + ("out", "partition_id"),
            out_names=("out",),
            lowering_input_output_aliases=(),
            sim_require_finite=True,
            sim_require_nnan=True,
            nc=nc,
        )
        return tuple(outs)

    devices = jax.devices()[:NB]
    mesh = Mesh(np.asarray(devices), ("core",))
    n_in = len(in_names) + 1  # + donated zero output
    fn = jax.jit(
        shard_map(
            _body,
            mesh=mesh,
            in_specs=(PartitionSpec("core"),) * n_in,
            out_specs=(PartitionSpec("core"),),
            check_rep=False,
        ),
        donate_argnums=(n_in - 1,),
        keep_unused=True,
    )
    _FAST = fn
    return fn


def _marshal(att_input, Wq, Wk, Wv):
    att_input = np.asarray(att_input, dtype=np.float32)
    # pre-transposed per-core x and shared weights (layout only, no FLOPs)
    xts = np.ascontiguousarray(att_input.transpose(0, 2, 1))  # [NB, D, S]
    wts = [
        np.ascontiguousarray(np.asarray(w, dtype=np.float32).T)
        for w in (Wq, Wk, Wv)
    ]
    if DIRECT and MM_MODE == "bf16":
        import ml_dtypes

        xts = xts.astype(ml_dtypes.bfloat16)
        wts = [w.astype(ml_dtypes.bfloat16) for w in wts]
    return xts, wts


def run(att_input, Wq, Wk, Wv, trace=False):
    xts, wts = _marshal(att_input, Wq, Wk, Wv)
    if trace:
        in_maps = [
            {"xt": xts[b], "wqt": wts[0], "wkt": wts[1], "wvt": wts[2]}
            for b in range(NB)
        ]
        res = bass_utils.run_bass_kernel_spmd(
            _get_nc(), in_maps, core_ids=list(range(NB)), trace=True
        )
        out = np.stack([res.results[b]["out"] for b in range(NB)], axis=0)
        return out.astype(np.float32, copy=False), res

    fn = _fast_runner()
    xs = xts.reshape(NB * D, S)
    ws = [np.concatenate([w] * NB, axis=0) for w in wts]
    zeros = np.zeros((NB * S, D), np.float32)
    (out,) = fn(xs, *ws, zeros)
    return np.asarray(out).reshape(NB, S, D).astype(np.float32, copy=False), None


def kernel(att_input, Wq, Wk, Wv):
    out, _ = run(att_input, Wq, Wk, Wv)
    return out
